# revision 9
# baseline (speedup 1.0000x reference)
"""Trainium2 Bass kernel: LeViT-style attention block (B=256, C=384, 14x14, 8 heads).

Data-parallel over batch: 32 images per NeuronCore, 8 cores.
Self-contained: takes full inputs, shards, runs SPMD, gathers full output.
"""
import os
import sys
import types

import numpy as np
import ml_dtypes

import concourse.bacc as bacc
import concourse.tile as tile
from concourse import mybir
from concourse.bass_utils import run_bass_kernel_spmd
from concourse.masks import make_identity

BF16 = ml_dtypes.bfloat16
EPS = 1e-5
NCORES = 8
B = 256
BPC = B // NCORES          # 32 images per core
PAIRS = BPC // 2
DIM, KEY_DIM, HEADS, RES = 384, 32, 8, 14
N = RES * RES              # 196
NH_KD, D, DH, H_QKV = 256, 128, 1024, 1536
SCALE = KEY_DIM ** -0.5
DT = mybir.dt
AF = mybir.ActivationFunctionType
OP = mybir.AluOpType

LAST_RESULT = None
_NC_CACHE = {}


def _install_ntff_hook():
    # The image's antenv lacks axon_hooks; synthesize it so trace=True (or
    # BASS_TRACE=1) yields exec_time_ns via the ctypes NTFF hook.
    try:
        import antenv
        from trn_agent_boot.trn_boot import _ntff_profile_via_ctypes

        if "antenv.axon_hooks" in sys.modules:
            return
        mod = types.ModuleType("antenv.axon_hooks")
        mod._hook = _ntff_profile_via_ctypes("/opt/axon/libaxon_pjrt.so")
        mod.set_axon_ntff_profile_hook = lambda h: setattr(mod, "_hook", h)
        mod.get_axon_ntff_profile_hook = lambda: mod._hook
        sys.modules["antenv.axon_hooks"] = mod
        antenv.axon_hooks = mod
    except Exception:
        pass


_install_ntff_hook()


def _build_nc():
    nc = bacc.Bacc("TRN2", target_bir_lowering=False, debug=False)
    x_d = nc.declare_dram_parameter("x", [BPC, DIM, N], DT.bfloat16, isOutput=False)
    qkvw_d = nc.declare_dram_parameter("qkv_wT", [DIM, H_QKV], DT.bfloat16, isOutput=False)
    dwdiag_d = nc.declare_dram_parameter("dw_diag", [2, 9, 128, 128], DT.bfloat16, isOutput=False)
    projw_d = nc.declare_dram_parameter("proj_wT", [DH, DIM], DT.bfloat16, isOutput=False)
    qkvb_d = nc.declare_dram_parameter("qkv_bias", [12, 128], DT.float32, isOutput=False)
    dwb_d = nc.declare_dram_parameter("dw_bias", [2, 128], DT.float32, isOutput=False)
    projb_d = nc.declare_dram_parameter("proj_bias", [3, 128], DT.float32, isOutput=False)
    ab_d = nc.declare_dram_parameter("ab", [HEADS, N, N], DT.bfloat16, isOutput=False)
    out_d = nc.declare_dram_parameter("out", [BPC, DIM, N], DT.float32, isOutput=True)

    from contextlib import ExitStack

    with tile.TileContext(nc) as tc, ExitStack() as es:
        const = es.enter_context(tc.tile_pool(name="const", bufs=1))
        xin = es.enter_context(tc.tile_pool(name="xin", bufs=2))
        stage = es.enter_context(tc.tile_pool(name="stage", bufs=2))
        sm = es.enter_context(tc.tile_pool(name="sm", bufs=4))
        att = es.enter_context(tc.tile_pool(name="att", bufs=10))
        outp = es.enter_context(tc.tile_pool(name="outp", bufs=3))
        psum = es.enter_context(tc.tile_pool(name="psum", bufs=8, space="PSUM"))

        # ---- constants ----
        qkvw_sb = const.tile([128, 3, H_QKV], DT.bfloat16)
        nc.sync.dma_start(qkvw_sb[:], qkvw_d.ap().rearrange("(k q) m -> q k m", q=128))
        projw_sb = const.tile([128, 8, DIM], DT.bfloat16)
        nc.sync.dma_start(projw_sb[:], projw_d.ap().rearrange("(k q) m -> q k m", q=128))
        dwdiag_sb = const.tile([128, 2, 9, 128], DT.bfloat16)
        nc.sync.dma_start(dwdiag_sb[:], dwdiag_d.ap().rearrange("c t q m -> q c t m"))
        qkvb_sb = const.tile([128, 12], DT.float32)
        nc.sync.dma_start(qkvb_sb[:], qkvb_d.ap().rearrange("m q -> q m"))
        dwb_sb = const.tile([128, 2], DT.float32)
        nc.sync.dma_start(dwb_sb[:], dwb_d.ap().rearrange("m q -> q m"))
        projb_sb = const.tile([128, 3], DT.float32)
        nc.sync.dma_start(projb_sb[:], projb_d.ap().rearrange("m q -> q m"))
        ab0_sb = const.tile([128, HEADS, N], DT.bfloat16)
        nc.sync.dma_start(ab0_sb[:], ab_d.ap()[:, 0:128].rearrange("h q m -> q h m"))
        ab1_sb = const.tile([68, HEADS, N], DT.bfloat16)
        nc.sync.dma_start(ab1_sb[:], ab_d.ap()[:, 128:196].rearrange("h q m -> q h m"))
        ident = const.tile([128, 128], DT.bfloat16)
        make_identity(nc, ident[:])

        for p in range(PAIRS):
            i0 = 2 * p
            # ---- load x pair: [c%128, kchunk, img, n] ----
            xt = xin.tile([128, 3, 2, N], DT.bfloat16)
            for k in range(3):
                nc.sync.dma_start(
                    xt[:, k],
                    x_d.ap()[i0:i0 + 2, 128 * k:128 * (k + 1)].rearrange("i q n -> q i n"),
                )

            qp = stage.tile([128, 2, 2, 16, 16], DT.bfloat16)  # padded q spatial
            nc.gpsimd.memset(qp[:], 0.0)
            k_sb = stage.tile([128, 2, 2, N], DT.bfloat16)
            v_sb = stage.tile([128, 8, 2, 256], DT.bfloat16)

            # ---- qkv 1x1 conv (+BN fold) ----
            for m in range(12):
                ps = psum.tile([128, 2, N], DT.float32, tag="ps")
                for k in range(3):
                    nc.tensor.matmul(
                        ps[:],
                        lhsT=qkvw_sb[:, k, 128 * m:128 * (m + 1)],
                        rhs=xt[:, k],
                        start=(k == 0),
                        stop=(k == 2),
                    )
                bias = qkvb_sb[:, m:m + 1]
                if m < 2:
                    nc.scalar.activation(
                        qp[:, m, :, 1:15, 1:15],
                        ps[:].rearrange("q i (y x) -> q i y x", y=RES),
                        AF.Identity,
                        bias=bias,
                    )
                elif m < 4:
                    nc.scalar.activation(k_sb[:, m - 2], ps[:], AF.Identity, bias=bias)
                else:
                    nc.scalar.activation(v_sb[:, m - 4, :, 0:N], ps[:], AF.Identity, bias=bias)

            # ---- depthwise 3x3 conv (+BN fold) as 9 diagonal matmuls ----
            qdw_sb = stage.tile([128, 2, 2, N], DT.bfloat16)
            for c2 in range(2):
                for img in range(2):
                    pd = psum.tile([128, RES, RES], DT.float32, tag="ps")
                    for tap in range(9):
                        dy, dx = divmod(tap, 3)
                        nc.tensor.matmul(
                            pd[:],
                            lhsT=dwdiag_sb[:, c2, tap],
                            rhs=qp[:, c2, img, dy:dy + 14, dx:dx + 14],
                            start=(tap == 0),
                            stop=(tap == 8),
                        )
                    nc.vector.tensor_scalar_add(
                        qdw_sb[:, c2, img].rearrange("q (y x) -> q y x", y=RES),
                        pd[:],
                        dwb_sb[:, c2:c2 + 1],
                    )

            # ---- attention per (img, head) ----
            # softmax(q^T k + ab) = exp(S)*exp_ab / rowsum; the multiply and the
            # rowsum fuse into one DVE scalar_tensor_tensor with accum_out.
            relu_sb = stage.tile([128, 8, 2, N], DT.bfloat16)
            for img in range(2):
                ssum = sm.tile([128, 8, 2], DT.float32)   # [n%128, h, nchunk]
                rinv = sm.tile([128, 8, 2], DT.float32)
                Ps = []
                for h in range(HEADS):
                    ch, sub = divmod(h, 4)
                    r0 = sub * 32
                    q_ap = qdw_sb[r0:r0 + 32, ch, img]   # [32, 196]
                    k_ap = k_sb[r0:r0 + 32, ch, img]     # [32, 196]

                    S = psum.tile([128, 2, N], DT.float32, tag="ps")
                    nc.tensor.matmul(S[:, 0], lhsT=q_ap[:, 0:128], rhs=k_ap,
                                     start=True, stop=True, tile_position=(r0, 0))
                    nc.tensor.matmul(S[:68, 1], lhsT=q_ap[:, 128:196], rhs=k_ap,
                                     start=True, stop=True, tile_position=(r0, 0))

                    # rows 68:128 of chunk 1 hold garbage; never read downstream
                    E = sm.tile([128, 2, N], DT.bfloat16)
                    nc.scalar.activation(E[:], S[:], AF.Exp)
                    P = att.tile([128, 2, 256], DT.bfloat16, tag="P")
                    nc.vector.scalar_tensor_tensor(
                        P[:, 0, 0:N], E[:, 0], 0.0, ab0_sb[:, h],
                        op0=OP.add, op1=OP.mult, accum_out=ssum[:, h, 0:1])
                    nc.vector.scalar_tensor_tensor(
                        P[:68, 1, 0:N], E[:68, 1], 0.0, ab1_sb[:, h],
                        op0=OP.add, op1=OP.mult, accum_out=ssum[:68, h, 1:2])
                    Ps.append(P)

                # one batched reciprocal per image
                nc.vector.reciprocal(rinv[:].rearrange("q a b -> q (a b)"),
                                     ssum[:].rearrange("q a b -> q (a b)"))

                for h in range(HEADS):
                    P = Ps[h]
                    Psc = att.tile([128, 2, 256], DT.bfloat16, tag="Psc")
                    nc.gpsimd.tensor_scalar_mul(Psc[:, 0, 0:N], P[:, 0, 0:N],
                                                rinv[:, h, 0:1])
                    nc.gpsimd.tensor_scalar_mul(Psc[:68, 1, 0:N], P[:68, 1, 0:N],
                                                rinv[:68, h, 1:2])

                    # P^T and v^T via DMA xbar transpose (pad regions unread)
                    PT = att.tile([128, 2, 208], DT.bfloat16, tag="PT")
                    nc.sync.dma_start_transpose(PT[:, 0, 0:128], Psc[:, 0, 0:128])
                    nc.sync.dma_start_transpose(PT[:, 1, 0:128], Psc[:, 0, 128:256])
                    nc.sync.dma_start_transpose(PT[:, 0, 128:208], Psc[0:80, 1, 0:128])
                    nc.sync.dma_start_transpose(PT[:, 1, 128:208], Psc[0:80, 1, 128:256])
                    vT = att.tile([128, 2, 128], DT.bfloat16, tag="vT")
                    nc.sync.dma_start_transpose(vT[:, 0], v_sb[:, h, img, 0:128])
                    nc.sync.dma_start_transpose(vT[:, 1], v_sb[:, h, img, 128:256])

                    # O = v @ P^T : [d, n]
                    O = psum.tile([128, N], DT.float32, tag="ps")
                    nc.tensor.matmul(O[:], lhsT=vT[:, 0], rhs=PT[:, 0, 0:N],
                                     start=True, stop=False)
                    nc.tensor.matmul(O[:], lhsT=vT[:68, 1], rhs=PT[:68, 1, 0:N],
                                     start=False, stop=True)
                    nc.vector.tensor_scalar_max(relu_sb[:, h, img], O[:], 0.0)

            # ---- proj 1x1 conv (+BN fold) ----
            for m3 in range(3):
                pp = psum.tile([128, 2, N], DT.float32, tag="ps")
                for k8 in range(8):
                    nc.tensor.matmul(
                        pp[:],
                        lhsT=projw_sb[:, k8, 128 * m3:128 * (m3 + 1)],
                        rhs=relu_sb[:, k8],
                        start=(k8 == 0),
                        stop=(k8 == 7),
                    )
                ob = outp.tile([128, 2, N], DT.float32)
                nc.vector.tensor_scalar_add(ob[:], pp[:], projb_sb[:, m3:m3 + 1])
                nc.sync.dma_start(
                    out_d.ap()[i0:i0 + 2, 128 * m3:128 * (m3 + 1)].rearrange("i q n -> q i n"),
                    ob[:],
                )

    nc.finalize()
    return nc


def _get_nc():
    if "nc" not in _NC_CACHE:
        _NC_CACHE["nc"] = _build_nc()
    return _NC_CACHE["nc"]


def _prep_host(x, qkv_w, qkv_g, qkv_b, qkv_m, qkv_v,
               dw_w, dw_g, dw_b, dw_m, dw_v,
               proj_w, proj_g, proj_b, proj_m, proj_v,
               attention_biases, bias_idxs):
    f = np.float32
    x = np.asarray(x, f)
    s = np.asarray(qkv_g, f) / np.sqrt(np.asarray(qkv_v, f) + EPS)
    W = np.asarray(qkv_w, f) * s[:, None]
    t = np.asarray(qkv_b, f) - np.asarray(qkv_m, f) * s
    # fold attention scale into k rows
    W[NH_KD:2 * NH_KD] *= SCALE
    t = t.copy()
    t[NH_KD:2 * NH_KD] *= SCALE
    qkv_wT = np.ascontiguousarray(W.T).astype(BF16)          # [384, 1536]
    qkv_bias = np.ascontiguousarray(t.reshape(12, 128))

    sd = np.asarray(dw_g, f) / np.sqrt(np.asarray(dw_v, f) + EPS)
    wd = np.asarray(dw_w, f)[:, 0] * sd[:, None, None]        # [256, 3, 3]
    td = np.asarray(dw_b, f) - np.asarray(dw_m, f) * sd
    dw_diag = np.zeros((2, 9, 128, 128), f)
    ii = np.arange(128)
    for c2 in range(2):
        for tap in range(9):
            dy, dx = divmod(tap, 3)
            dw_diag[c2, tap, ii, ii] = wd[c2 * 128:(c2 + 1) * 128, dy, dx]
    dw_diag = dw_diag.astype(BF16)
    dw_bias = np.ascontiguousarray(td.reshape(2, 128))

    sp = np.asarray(proj_g, f) / np.sqrt(np.asarray(proj_v, f) + EPS)
    Wp = np.asarray(proj_w, f) * sp[:, None]
    tp = np.asarray(proj_b, f) - np.asarray(proj_m, f) * sp
    proj_wT = np.ascontiguousarray(Wp.T).astype(BF16)         # [1024, 384]
    proj_bias = np.ascontiguousarray(tp.reshape(3, 128))

    ab = np.asarray(attention_biases, f)[:, np.asarray(bias_idxs)]  # [8, 196, 196]
    ab = np.ascontiguousarray(np.exp(ab)).astype(BF16)

    x_bf = np.ascontiguousarray(x.reshape(B, DIM, N)).astype(BF16)
    return x_bf, dict(qkv_wT=qkv_wT, dw_diag=dw_diag, proj_wT=proj_wT,
                      qkv_bias=qkv_bias, dw_bias=dw_bias, proj_bias=proj_bias, ab=ab)


def kernel(**inputs):
    global LAST_RESULT
    x_bf, consts = _prep_host(**inputs)
    nc = _get_nc()
    in_maps = []
    for c in range(NCORES):
        m = {"x": np.ascontiguousarray(x_bf[c * BPC:(c + 1) * BPC])}
        m.update(consts)
        in_maps.append(m)
    res = run_bass_kernel_spmd(nc, in_maps, core_ids=list(range(NCORES)))
    LAST_RESULT = res
    out = np.concatenate([r["out"] for r in res.results], axis=0)
    return np.ascontiguousarray(out.reshape(B, DIM, RES, RES)).astype(np.float32)


# revision 10
# speedup vs baseline: 2.7342x; 2.7342x over previous
"""Trainium2 Bass kernel: LeViT-style attention block (B=256, C=384, 14x14, 8 heads).

Data-parallel over batch: 32 images per NeuronCore, 8 cores.
Self-contained: takes full inputs, shards, runs SPMD, gathers full output.
"""
import os
import sys
import types

import numpy as np
import ml_dtypes

import concourse.bacc as bacc
import concourse.tile as tile
from concourse import mybir
from concourse.bass_utils import run_bass_kernel_spmd
from concourse.masks import make_identity

BF16 = ml_dtypes.bfloat16
EPS = 1e-5
NCORES = 8
B = 256
BPC = B // NCORES          # 32 images per core
PAIRS = BPC // 2
DIM, KEY_DIM, HEADS, RES = 384, 32, 8, 14
N = RES * RES              # 196
NH_KD, D, DH, H_QKV = 256, 128, 1024, 1536
SCALE = KEY_DIM ** -0.5
DT = mybir.dt
AF = mybir.ActivationFunctionType
OP = mybir.AluOpType

LAST_RESULT = None
_NC_CACHE = {}


def _install_ntff_hook():
    # The image's antenv lacks axon_hooks; synthesize it so trace=True (or
    # BASS_TRACE=1) yields exec_time_ns via the ctypes NTFF hook.
    try:
        import antenv
        from trn_agent_boot.trn_boot import _ntff_profile_via_ctypes

        if "antenv.axon_hooks" in sys.modules:
            return
        mod = types.ModuleType("antenv.axon_hooks")
        mod._hook = _ntff_profile_via_ctypes("/opt/axon/libaxon_pjrt.so")
        mod.set_axon_ntff_profile_hook = lambda h: setattr(mod, "_hook", h)
        mod.get_axon_ntff_profile_hook = lambda: mod._hook
        sys.modules["antenv.axon_hooks"] = mod
        antenv.axon_hooks = mod
    except Exception:
        pass


_install_ntff_hook()


def _build_nc():
    nc = bacc.Bacc("TRN2", target_bir_lowering=False, debug=False)
    x_d = nc.declare_dram_parameter("x", [BPC, DIM, N], DT.bfloat16, isOutput=False)
    qkvw_d = nc.declare_dram_parameter("qkv_wT", [DIM, H_QKV], DT.bfloat16, isOutput=False)
    dwdiag_d = nc.declare_dram_parameter("dw_diag", [2, 9, 128, 128], DT.bfloat16, isOutput=False)
    projw_d = nc.declare_dram_parameter("proj_wT", [DH, DIM], DT.bfloat16, isOutput=False)
    qkvb_d = nc.declare_dram_parameter("qkv_bias", [12, 128], DT.float32, isOutput=False)
    dwb_d = nc.declare_dram_parameter("dw_bias", [2, 128], DT.float32, isOutput=False)
    projb_d = nc.declare_dram_parameter("proj_bias", [3, 128], DT.float32, isOutput=False)
    ab_d = nc.declare_dram_parameter("ab", [HEADS, N, N], DT.bfloat16, isOutput=False)
    out_d = nc.declare_dram_parameter("out", [BPC, DIM, N], DT.float32, isOutput=True)

    from contextlib import ExitStack

    with tile.TileContext(nc) as tc, ExitStack() as es:
        const = es.enter_context(tc.tile_pool(name="const", bufs=1))
        xin = es.enter_context(tc.tile_pool(name="xin", bufs=2))
        stage = es.enter_context(tc.tile_pool(name="stage", bufs=2))
        sm = es.enter_context(tc.tile_pool(name="sm", bufs=4))
        att = es.enter_context(tc.tile_pool(name="att", bufs=10))
        outp = es.enter_context(tc.tile_pool(name="outp", bufs=3))
        psum = es.enter_context(tc.tile_pool(name="psum", bufs=8, space="PSUM"))

        # ---- constants ----
        qkvw_sb = const.tile([128, 3, H_QKV], DT.bfloat16)
        nc.sync.dma_start(qkvw_sb[:], qkvw_d.ap().rearrange("(k q) m -> q k m", q=128))
        projw_sb = const.tile([128, 8, DIM], DT.bfloat16)
        nc.sync.dma_start(projw_sb[:], projw_d.ap().rearrange("(k q) m -> q k m", q=128))
        dwdiag_sb = const.tile([128, 2, 9, 128], DT.bfloat16)
        nc.sync.dma_start(dwdiag_sb[:], dwdiag_d.ap().rearrange("c t q m -> q c t m"))
        qkvb_sb = const.tile([128, 12], DT.float32)
        nc.sync.dma_start(qkvb_sb[:], qkvb_d.ap().rearrange("m q -> q m"))
        dwb_sb = const.tile([128, 2], DT.float32)
        nc.sync.dma_start(dwb_sb[:], dwb_d.ap().rearrange("m q -> q m"))
        projb_sb = const.tile([128, 3], DT.float32)
        nc.sync.dma_start(projb_sb[:], projb_d.ap().rearrange("m q -> q m"))
        ab0_sb = const.tile([128, HEADS, N], DT.bfloat16)
        nc.sync.dma_start(ab0_sb[:], ab_d.ap()[:, 0:128].rearrange("h q m -> q h m"))
        ab1_sb = const.tile([68, HEADS, N], DT.bfloat16)
        nc.sync.dma_start(ab1_sb[:], ab_d.ap()[:, 128:196].rearrange("h q m -> q h m"))
        ident = const.tile([128, 128], DT.bfloat16)
        make_identity(nc, ident[:])

        for p in range(PAIRS):
            i0 = 2 * p
            # ---- load x pair: [c%128, kchunk, img, n] ----
            xt = xin.tile([128, 3, 2, N], DT.bfloat16)
            for k in range(3):
                nc.sync.dma_start(
                    xt[:, k],
                    x_d.ap()[i0:i0 + 2, 128 * k:128 * (k + 1)].rearrange("i q n -> q i n"),
                )

            qp = stage.tile([128, 2, 2, 16, 16], DT.bfloat16)  # padded q spatial
            nc.gpsimd.memset(qp[:], 0.0)
            k_sb = stage.tile([128, 2, 2, N], DT.bfloat16)

            # ---- qkv 1x1 conv for q,k only (+BN fold); v is produced
            # transposed directly from x below ----
            for m in range(4):
                ps = psum.tile([128, 2, N], DT.float32, tag="ps")
                for k in range(3):
                    nc.tensor.matmul(
                        ps[:],
                        lhsT=qkvw_sb[:, k, 128 * m:128 * (m + 1)],
                        rhs=xt[:, k],
                        start=(k == 0),
                        stop=(k == 2),
                    )
                bias = qkvb_sb[:, m:m + 1]
                if m < 2:
                    nc.scalar.activation(
                        qp[:, m, :, 1:15, 1:15],
                        ps[:].rearrange("q i (y x) -> q i y x", y=RES),
                        AF.Identity,
                        bias=bias,
                    )
                else:
                    nc.scalar.activation(k_sb[:, m - 2], ps[:], AF.Identity, bias=bias)

            # ---- v^T produced directly: vT[m, dv] = x^T @ Wv^T ----
            # (v's BN bias is applied later at the O eviction: rows of the
            # normalized attention sum to 1, so (v+tv)@P^T = v@P^T + tv)
            vTs = []
            for img in range(2):
                vT_sb = stage.tile([128, 2, DH], DT.bfloat16, tag="vT_sb")
                for mc in range(2):
                    m_lo, m_sz = (0, 128) if mc == 0 else (128, 68)
                    for half in range(2):
                        pv = psum.tile([128, 512], DT.float32, tag="ps")
                        for k in range(3):
                            nc.tensor.matmul(
                                pv[0:m_sz],
                                lhsT=xt[:, k, img, m_lo:m_lo + m_sz],
                                rhs=qkvw_sb[:, k, 512 + 512 * half:512 + 512 * (half + 1)],
                                start=(k == 0),
                                stop=(k == 2),
                            )
                        eng = nc.scalar if half == 0 else nc.vector
                        if half == 0:
                            nc.scalar.activation(
                                vT_sb[0:m_sz, mc, 0:512], pv[0:m_sz], AF.Copy)
                        else:
                            nc.vector.tensor_copy(
                                vT_sb[0:m_sz, mc, 512:1024], pv[0:m_sz])
                vTs.append(vT_sb)

            # ---- depthwise 3x3 conv (+BN fold) as 9 diagonal matmuls ----
            qdw_sb = stage.tile([128, 2, 2, N], DT.bfloat16)
            for c2 in range(2):
                for img in range(2):
                    pd = psum.tile([128, RES, RES], DT.float32, tag="ps")
                    for tap in range(9):
                        dy, dx = divmod(tap, 3)
                        nc.tensor.matmul(
                            pd[:],
                            lhsT=dwdiag_sb[:, c2, tap],
                            rhs=qp[:, c2, img, dy:dy + 14, dx:dx + 14],
                            start=(tap == 0),
                            stop=(tap == 8),
                        )
                    nc.vector.tensor_scalar_add(
                        qdw_sb[:, c2, img].rearrange("q (y x) -> q y x", y=RES),
                        pd[:],
                        dwb_sb[:, c2:c2 + 1],
                    )

            # ---- attention per (img, head) ----
            # softmax(q^T k + ab) = exp(S)*exp_ab / rowsum; the multiply and the
            # rowsum fuse into one DVE scalar_tensor_tensor with accum_out.
            relu_sb = stage.tile([128, 8, 2, N], DT.bfloat16)
            for img in range(2):
                ssum = sm.tile([128, 8, 2], DT.float32)   # [n%128, h, nchunk]
                rinv = sm.tile([128, 8, 2], DT.float32)
                Ps = []
                for h in range(HEADS):
                    ch, sub = divmod(h, 4)
                    r0 = sub * 32
                    q_ap = qdw_sb[r0:r0 + 32, ch, img]   # [32, 196]
                    k_ap = k_sb[r0:r0 + 32, ch, img]     # [32, 196]

                    S = psum.tile([128, 2, N], DT.float32, tag="ps")
                    nc.tensor.matmul(S[:, 0], lhsT=q_ap[:, 0:128], rhs=k_ap,
                                     start=True, stop=True, tile_position=(r0, 0))
                    nc.tensor.matmul(S[:68, 1], lhsT=q_ap[:, 128:196], rhs=k_ap,
                                     start=True, stop=True, tile_position=(r0, 0))

                    # rows 68:128 of chunk 1 hold garbage; never read downstream
                    E = sm.tile([128, 2, N], DT.bfloat16)
                    nc.scalar.activation(E[:], S[:], AF.Exp)
                    P = att.tile([128, 2, 256], DT.bfloat16, tag="P")
                    nc.vector.scalar_tensor_tensor(
                        P[:, 0, 0:N], E[:, 0], 0.0, ab0_sb[:, h],
                        op0=OP.add, op1=OP.mult, accum_out=ssum[:, h, 0:1])
                    nc.vector.scalar_tensor_tensor(
                        P[:68, 1, 0:N], E[:68, 1], 0.0, ab1_sb[:, h],
                        op0=OP.add, op1=OP.mult, accum_out=ssum[:68, h, 1:2])
                    Ps.append(P)

                # one batched reciprocal per image
                nc.vector.reciprocal(rinv[:].rearrange("q a b -> q (a b)"),
                                     ssum[:].rearrange("q a b -> q (a b)"))

                vT_sb = vTs[img]
                for h in range(HEADS):
                    P = Ps[h]
                    Psc = att.tile([128, 2, N], DT.bfloat16, tag="Psc")
                    nc.vector.tensor_scalar_mul(Psc[:, 0], P[:, 0, 0:N],
                                                rinv[:, h, 0:1])
                    nc.vector.tensor_scalar_mul(Psc[:68, 1], P[:68, 1, 0:N],
                                                rinv[:68, h, 1:2])

                    # P^T via PE transpose (psum), evictions split ACT/DVE
                    PTp = psum.tile([128, 2, N], DT.bfloat16, tag="ps")
                    nc.tensor.transpose(PTp[:, 0, 0:128], Psc[:, 0, 0:128], ident[:])
                    nc.tensor.transpose(PTp[:68, 1, 0:128], Psc[:, 0, 128:196], ident[:])
                    nc.tensor.transpose(PTp[:, 0, 128:196], Psc[:68, 1, 0:128], ident[:68, :68])
                    nc.tensor.transpose(PTp[:68, 1, 128:196], Psc[:68, 1, 128:196], ident[:68, :68])
                    PT = att.tile([128, 2, N], DT.bfloat16, tag="PT")
                    nc.scalar.activation(PT[:, 0], PTp[:, 0], AF.Copy)
                    nc.vector.tensor_copy(PT[:68, 1], PTp[:68, 1])

                    # O = v @ P^T : [d, n]; +tv bias and relu fused on eviction
                    O = psum.tile([128, N], DT.float32, tag="ps")
                    nc.tensor.matmul(O[:], lhsT=vT_sb[:, 0, 128 * h:128 * (h + 1)],
                                     rhs=PT[:, 0], start=True, stop=False)
                    nc.tensor.matmul(O[:], lhsT=vT_sb[:68, 1, 128 * h:128 * (h + 1)],
                                     rhs=PT[:68, 1], start=False, stop=True)
                    nc.scalar.activation(relu_sb[:, h, img], O[:], AF.Relu,
                                         bias=qkvb_sb[:, 4 + h:5 + h])

            # ---- proj 1x1 conv (+BN fold) ----
            for m3 in range(3):
                pp = psum.tile([128, 2, N], DT.float32, tag="ps")
                for k8 in range(8):
                    nc.tensor.matmul(
                        pp[:],
                        lhsT=projw_sb[:, k8, 128 * m3:128 * (m3 + 1)],
                        rhs=relu_sb[:, k8],
                        start=(k8 == 0),
                        stop=(k8 == 7),
                    )
                ob = outp.tile([128, 2, N], DT.float32)
                nc.vector.tensor_scalar_add(ob[:], pp[:], projb_sb[:, m3:m3 + 1])
                nc.sync.dma_start(
                    out_d.ap()[i0:i0 + 2, 128 * m3:128 * (m3 + 1)].rearrange("i q n -> q i n"),
                    ob[:],
                )

    nc.finalize()
    return nc


def _get_nc():
    if "nc" not in _NC_CACHE:
        _NC_CACHE["nc"] = _build_nc()
    return _NC_CACHE["nc"]


def _prep_host(x, qkv_w, qkv_g, qkv_b, qkv_m, qkv_v,
               dw_w, dw_g, dw_b, dw_m, dw_v,
               proj_w, proj_g, proj_b, proj_m, proj_v,
               attention_biases, bias_idxs):
    f = np.float32
    x = np.asarray(x, f)
    s = np.asarray(qkv_g, f) / np.sqrt(np.asarray(qkv_v, f) + EPS)
    W = np.asarray(qkv_w, f) * s[:, None]
    t = np.asarray(qkv_b, f) - np.asarray(qkv_m, f) * s
    # fold attention scale into k rows
    W[NH_KD:2 * NH_KD] *= SCALE
    t = t.copy()
    t[NH_KD:2 * NH_KD] *= SCALE
    qkv_wT = np.ascontiguousarray(W.T).astype(BF16)          # [384, 1536]
    qkv_bias = np.ascontiguousarray(t.reshape(12, 128))

    sd = np.asarray(dw_g, f) / np.sqrt(np.asarray(dw_v, f) + EPS)
    wd = np.asarray(dw_w, f)[:, 0] * sd[:, None, None]        # [256, 3, 3]
    td = np.asarray(dw_b, f) - np.asarray(dw_m, f) * sd
    dw_diag = np.zeros((2, 9, 128, 128), f)
    ii = np.arange(128)
    for c2 in range(2):
        for tap in range(9):
            dy, dx = divmod(tap, 3)
            dw_diag[c2, tap, ii, ii] = wd[c2 * 128:(c2 + 1) * 128, dy, dx]
    dw_diag = dw_diag.astype(BF16)
    dw_bias = np.ascontiguousarray(td.reshape(2, 128))

    sp = np.asarray(proj_g, f) / np.sqrt(np.asarray(proj_v, f) + EPS)
    Wp = np.asarray(proj_w, f) * sp[:, None]
    tp = np.asarray(proj_b, f) - np.asarray(proj_m, f) * sp
    proj_wT = np.ascontiguousarray(Wp.T).astype(BF16)         # [1024, 384]
    proj_bias = np.ascontiguousarray(tp.reshape(3, 128))

    ab = np.asarray(attention_biases, f)[:, np.asarray(bias_idxs)]  # [8, 196, 196]
    ab = np.ascontiguousarray(np.exp(ab)).astype(BF16)

    x_bf = np.ascontiguousarray(x.reshape(B, DIM, N)).astype(BF16)
    return x_bf, dict(qkv_wT=qkv_wT, dw_diag=dw_diag, proj_wT=proj_wT,
                      qkv_bias=qkv_bias, dw_bias=dw_bias, proj_bias=proj_bias, ab=ab)


def kernel(**inputs):
    global LAST_RESULT
    x_bf, consts = _prep_host(**inputs)
    nc = _get_nc()
    in_maps = []
    for c in range(NCORES):
        m = {"x": np.ascontiguousarray(x_bf[c * BPC:(c + 1) * BPC])}
        m.update(consts)
        in_maps.append(m)
    res = run_bass_kernel_spmd(nc, in_maps, core_ids=list(range(NCORES)))
    LAST_RESULT = res
    out = np.concatenate([r["out"] for r in res.results], axis=0)
    return np.ascontiguousarray(out.reshape(B, DIM, RES, RES)).astype(np.float32)
